# revision 54
# baseline (speedup 1.0000x reference)
"""CLIP-style attention with MULT-expanded K/V (nn_CLIPAttentionMKV) on 8
Trainium2 NeuronCores.

Sharding: core = (batch b, head-group g); 4 batches x 2 groups of 8 heads.
Each core computes its batch's Q/K/V projections for its 8 heads, the
per-head attention, and a partial output projection (contracting over its
512 of the 1024 hidden features).  Host sums the two partials per batch.

x and the Q/K/V weights ship as fp8e4 (hi, lo-residual) pairs with
power-of-2 scales; the projections run as 3-term fp8 DoubleRow matmuls
(hi*hi + lo*hi + hi*lo, two k-tiles per instruction at 0.5 cycles/row),
which both quarters the per-MAC tensor-engine cost and beats bf16
accuracy.  Q/K land in SBUF as float32r; the value path (exp scores, V,
attention out, Wo) is bfloat16.

The AV matmul is oriented with the exp-scores *stationary* and V
*moving*, so each instruction streams 65 rows (64 head dims + an
all-ones column that accumulates the softmax normalizer Z) instead of
512 query columns -- half the tensor-engine rows of the natural
orientation.  The 8 AV accumulation chains per (tau, head-pair) window
live as sub-bank regions of two PSUM banks, exploiting the per-element
has_written overwrite-then-accumulate semantics (only the first matmul
of a bank issues start=True, which clears the whole bank).  Chains
drain with a strided DVE reciprocal of the Z column plus one
tensor_scalar multiply per query-chunk (AV * 1/Z -> bf16).  The
[token, feature] attention output is PE-transposed (identity matmul)
into [feature, token] for the bf16 output projection.

Scheduling: the exp stream (128 x ~1.05us on ACT) paces the attention
windows, so the emitter software-pipelines each step -- QK/exp for step
i+1 and the just-in-time fp8 V-projection chain are emitted BEFORE the
exp-gated AV matmuls of step i -- and pumps ~450ns of pending
projection/transpose/output-projection units from an ordered filler
queue per step, keeping the tensor engine >95% busy from the first
microseconds to the tail.  Window order (0,0),(0,1),(1,0),(1,1),(0,2),
(0,3),(1,2),(1,3) lets the no-new-projection tau1 windows absorb the
tau0 output projection and the later K projections.  x arrives
per-k-tile interleaved across both HWDGE queues so the first chains
consume it progressively (warming the PE p-state), small tensors ride
the gpsimd SWDGE queue, and output DMAs split across both queues at
the tail.
"""

import numpy as np

import concourse.bacc as bacc
import concourse.bass as bass
import concourse.mybir as mybir
import concourse.tile as tile
from concourse import bass_utils
from concourse.bass import ts

B, T, E = 4, 1024, 1024
H, MULT = 16, 2
HD = E // H            # 64
S = T * MULT           # 2048
SCALE = HD ** -0.5
P = 128
G = 2                  # head groups == cores per batch
HG = H // G            # 8 heads per group
FG = HG * HD           # 512 q features per group
F2 = MULT * FG         # 1024 k features per group
FV = MULT * HG * (HD + 1)   # 1040 augmented v features per group
FCH = FV // 4          # 260: v-proj chunk
N_CORES = B * G
NT = 512               # matmul moving free dim
KO = E // P            # 8 contraction k-tiles for projections

F32 = mybir.dt.float32
F32R = mybir.dt.float32r
BF16 = mybir.dt.bfloat16
FP8 = mybir.dt.float8e4
DR = mybir.MatmulPerfMode.DoubleRow
ADD = mybir.AluOpType.add
MUL = mybir.AluOpType.mult
EXP = mybir.ActivationFunctionType.Exp

_compiled = {}


def _build(aug):
    nc = bacc.Bacc("TRN2", target_bir_lowering=False, debug=False,
                   num_devices=N_CORES)
    # x and the q/k/v weights ship as fp8e4 (hi, lo-residual) pairs packed
    # row-interleaved (row = ko*2P + p*2 + half); the inverse scales ride
    # in wsc.
    xT = nc.dram_tensor("xT", [2 * E, T], FP8, kind="ExternalInput").ap()
    wq = nc.dram_tensor("wq", [2 * E, FG], FP8, kind="ExternalInput").ap()
    wk = nc.dram_tensor("wk", [2 * E, F2], FP8, kind="ExternalInput").ap()
    wv = nc.dram_tensor("wv", [2 * E, FV if aug else F2], FP8,
                        kind="ExternalInput").ap()
    wo = nc.dram_tensor("wo", [FG, E], BF16, kind="ExternalInput").ap()
    wsc = nc.dram_tensor("wsc", [P, 4], F32, kind="ExternalInput").ap()
    bq = nc.dram_tensor("bq", [FG], F32, kind="ExternalInput").ap()
    bk = nc.dram_tensor("bk", [F2], F32, kind="ExternalInput").ap()
    if aug:
        bv = nc.dram_tensor("bv", [FV], BF16, kind="ExternalInput").ap()
    bo = nc.dram_tensor("bo", [E], F32, kind="ExternalInput").ap()
    ident = nc.dram_tensor("ident", [P, P], BF16, kind="ExternalInput").ap()
    if aug:
        ones = nc.dram_tensor("ones", [P], BF16,
                              kind="ExternalInput").ap()
    out = nc.dram_tensor("out", [E, T], BF16, kind="ExternalOutput").ap()

    with tile.TileContext(nc) as tc:
        with (
            tc.tile_pool(name="resident", bufs=1) as res,
            # one PSUM pool; tags get disjoint bank-aligned slots:
            # qk 2x2 + av0 1 + av1 1 + mm 2x1 = 8 banks exactly.
            tc.tile_pool(name="psum", bufs=1, space="PSUM") as psum,
            tc.tile_pool(name="wpool", bufs=1) as wp,
            tc.tile_pool(name="epool", bufs=3) as ep,
            tc.tile_pool(name="osb", bufs=3) as ob,
        ):
            # ---- resident tiles ----
            q_sb = res.tile([P, FG // P, T], F32R)      # q^T  [f, t]
            kfeat = res.tile([P, F2 // P, T], F32R)     # k^T  [f, t]
            vaug = res.tile([P, T // P, FV], BF16)      # v    [t, faug]
            a_sb = res.tile([P, T // P, FG], BF16)      # attn [t, f]
            attn_out = res.tile([P, FG // P, T], BF16)  # attn^T [f, t]
            ident_sb = res.tile([P, P], BF16)
            if aug:
                ones_1 = res.tile([1, P], BF16)     # K=1 bias-row lhsT
            bq_sb = res.tile([P, FG // P], F32)
            bk_sb = res.tile([P, F2 // P], F32)
            if aug:
                bv_sb = res.tile([1, FV], BF16)
            bo_sb = res.tile([P, E // P], F32)
            wsc_sb = res.tile([P, 4], F32)

            wq3 = wq.rearrange("(p kt) f -> p kt f", p=P)
            wk3 = wk.rearrange("(p kt) f -> p kt f", p=P)
            wv3 = wv.rearrange("(p kt) f -> p kt f", p=P)
            xT3 = xT.rearrange("(p kt) t -> p kt t", p=P)
            wo3 = wo.rearrange("(ko p) f -> p ko f", p=P)
            out3 = out.rearrange("(jo p) t -> p jo t", p=P)

            xT_sb = res.tile([P, KO * 2, T], FP8)
            x4 = xT_sb.rearrange("p (ko two) t -> p ko two t", two=2)

            # ---------- upfront DMAs on both HWDGE queues ----------

            vch = FCH if aug else NT // 2

            def w_dma(eng, kind, j):
                if kind == "q":
                    t_ = wp.tile([P, KO * 2, P], FP8, tag="wq", bufs=4,
                                 name=f"wtq_{j}")
                    eng.dma_start(t_[:], wq3[:, :, ts(j, P)])
                elif kind == "k":
                    t_ = wp.tile([P, KO * 2, P], FP8, tag="wk", bufs=8,
                                 name=f"wtk_{j}")
                    eng.dma_start(t_[:], wk3[:, :, ts(j, P)])
                else:
                    t_ = wp.tile([P, KO * 2, vch], FP8, tag="wv", bufs=4,
                                 name=f"wtv_{j}")
                    eng.dma_start(t_[:], wv3[:, :, ts(j, vch)])
                return t_.rearrange("p (ko two) f -> p ko two f", two=2)

            wt_q, wt_k, wt_v = {}, {}, {}
            # x is interleaved per-ko across both HWDGE queues, behind the
            # first window's weight tiles, so the q0/k0 chains consume the
            # k-tiles progressively as they land (PE warms up and the first
            # exp fires ~10us in).
            wt_q[0] = w_dma(nc.scalar, "q", 0)
            wt_k[0] = w_dma(nc.sync, "k", 0)
            for ko in range(0, KO, 2):
                nc.sync.dma_start(xT_sb[:, 2 * ko:2 * ko + 2],
                                  xT3[:, 2 * ko:2 * ko + 2])
                nc.scalar.dma_start(xT_sb[:, 2 * ko + 2:2 * ko + 4],
                                    xT3[:, 2 * ko + 2:2 * ko + 4])
            nc.gpsimd.dma_start(bq_sb[:], bq.rearrange("(o p) -> p o", p=P))
            nc.gpsimd.dma_start(bk_sb[:], bk.rearrange("(o p) -> p o", p=P))
            nc.gpsimd.dma_start(wsc_sb[:], wsc[:, :])
            wt_v[0] = w_dma(nc.scalar, "v", 0)
            nc.gpsimd.dma_start(ident_sb[:], ident[:, :])
            if aug:
                nc.gpsimd.dma_start(ones_1[:], ones[None, :])
                nc.gpsimd.dma_start(bv_sb[:], bv[None, :])
            wt_k[4] = w_dma(nc.sync, "k", 4)
            wt_v[2] = w_dma(nc.sync, "v", 2)
            wt_q[1] = w_dma(nc.sync, "q", 1)
            wt_k[1] = w_dma(nc.sync, "k", 1)
            wt_k[5] = w_dma(nc.sync, "k", 5)
            wt_v[1] = w_dma(nc.sync, "v", 1)
            wt_q[2] = w_dma(nc.sync, "q", 2)
            wt_k[2] = w_dma(nc.sync, "k", 2)
            wt_k[6] = w_dma(nc.sync, "k", 6)
            wt_v[3] = w_dma(nc.sync, "v", 3)
            wt_q[3] = w_dma(nc.sync, "q", 3)
            wt_k[3] = w_dma(nc.sync, "k", 3)
            wt_k[7] = w_dma(nc.sync, "k", 7)
            wots = []
            for j in range(E // P):
                wot = wp.tile([P, FG // P, P], BF16, tag="wo", bufs=16,
                              name=f"wot_{j}")
                nc.sync.dma_start(wot[:], wo3[:, :, ts(j, P)])
                wots.append(wot)
            nc.sync.dma_start(bo_sb[:], bo.rearrange("(o p) -> p o", p=P))

            if not aug:
                onesf = res.tile([P, T // P, MULT * HG], F32, name="onesf")
                nc.gpsimd.memset(onesf[:], 1.0)
                va5 = vaug.rearrange("p i (b c) -> p i b c", c=HD + 1)
                nc.vector.tensor_copy(va5[:, :, :, HD:HD + 1], onesf[:])

            # p-state warmup: ~3us of dependency-free junk matmuls while the
            # xT DMA is in flight, so the first real chains run at full clock.
            zt = res.tile([32, NT], BF16, name="zt")
            nc.gpsimd.memset(zt[:], 0.0)
            for w in range(4):
                wpt = psum.tile([32, NT], F32, tag="mm", bufs=2,
                                name=f"warm_{w}")
                nc.tensor.matmul(wpt[:], zt[:, 0:32], zt[:],
                                 start=True, stop=True)

            # ---------- chain emitters ----------
            # fp8 3-term residual product: hi*hi + lo*hi + hi*lo, each step
            # a DoubleRow matmul over a ko-pair (contraction 256/instr).
            TERMS = [(0, 0), (0, 1), (1, 0)]   # (w half, x half)
            NSTEP = 3 * KO // 2                # 12 steps per chain

            def qk_chain(kind, j, tau, ptag="mm", pbufs=2):
                """One projection chain: [P, NT] psum accumulated over 12
                fp8 DoubleRow steps, then DVE scale+bias into the output."""
                wt = (wt_q if kind == "q" else wt_k)[j]
                b_sb = bq_sb if kind == "q" else bk_sb
                o_sb = q_sb if kind == "q" else kfeat
                sc = 0 if kind == "q" else 1
                pt = psum.tile([P, NT], F32, tag=ptag, bufs=pbufs,
                               name=f"pj_{kind}_{j}_{tau}")

                def piece(s0, n):
                    for s in range(s0, s0 + n):
                        wi, xi = TERMS[s // (KO // 2)]
                        kp = 2 * (s % (KO // 2))
                        nc.tensor.matmul(
                            pt[:], wt[:, kp:kp + 2, wi, :],
                            x4[:, kp:kp + 2, xi, ts(tau, NT)],
                            start=(s == 0), stop=(s == NSTEP - 1),
                            perf_mode=DR)
                    if s0 + n == NSTEP:
                        nc.vector.scalar_tensor_tensor(
                            o_sb[:, j, ts(tau, NT)], pt[:],
                            wsc_sb[:, sc:sc + 1],
                            b_sb[:, j:j + 1].to_broadcast((P, NT)),
                            MUL, ADD)
                return piece

            def v_chain(phi, i):
                """One v-projection chain: [P, vch] psum for token block i,
                scaled and scattered into the 65-stride bf16 layout."""
                wvt = wt_v[phi]
                pt = psum.tile([P, vch], F32, tag="mm", bufs=2,
                               name=f"pv_{phi}_{i}")

                def piece(s0, n):
                    for s in range(s0, s0 + n):
                        wi, xi = TERMS[s // (KO // 2)]
                        kp = 2 * (s % (KO // 2))
                        nc.tensor.matmul(
                            pt[:], x4[:, kp:kp + 2, xi, ts(i, P)],
                            wvt[:, kp:kp + 2, wi, :],
                            start=(s == 0),
                            stop=(False if aug else s == NSTEP - 1),
                            perf_mode=DR)
                    if s0 + n == NSTEP:
                        if aug:
                            nc.tensor.matmul(
                                pt[:], ones_1[:], bv_sb[:, ts(phi, FCH)],
                                start=False, stop=True)
                            nc.vector.tensor_scalar(
                                vaug[:, i, ts(phi, FCH)], pt[:],
                                wsc_sb[:, 2:3], None, MUL)
                        else:
                            dst = vaug[:, i, ts(phi, FCH)].rearrange(
                                "p (b c) -> p b c", c=HD + 1)
                            nc.vector.tensor_scalar(
                                dst[:, :, 0:HD],
                                pt.rearrange("p (b c) -> p b c", c=HD),
                                wsc_sb[:, 2:3], None, MUL)
                return piece

            def transp(tc_, fo):
                tp = psum.tile([P, P], BF16, tag="mm", bufs=2,
                               name=f"tp_{tc_}_{fo}")
                nc.tensor.matmul(tp[:], a_sb[:, tc_, ts(fo, P)],
                                 ident_sb[:], is_transpose=True)
                nc.vector.tensor_copy(attn_out[:, fo, ts(tc_, P)], tp[:])

            def op_chain(tau, j, tg="mm", tb=2):
                pt = psum.tile([P, NT], F32, tag=tg, bufs=tb,
                               name=f"op_{j}_{tau}")

                def piece(ko0, n):
                    for ko in range(ko0, ko0 + n):
                        nc.tensor.matmul(
                            pt[:], wots[j][:, ko],
                            attn_out[:, ko, ts(tau, NT)],
                            start=(ko == 0), stop=(ko == FG // P - 1))
                    if ko0 + n == FG // P:
                        ot = ob.tile([P, NT], BF16, tag="ot", bufs=3,
                                     name=f"ot_{j}_{tau}")
                        nc.vector.tensor_tensor(
                            ot[:], pt[:],
                            bo_sb[:, j:j + 1].to_broadcast((P, NT)), ADD)
                        # tau0 DMAs go mid-attention: keep them off the ACT
                        # sequencer (it is dispatching exps).  tau1 runs at
                        # the tail where the ACT queue is free.
                        eng = nc.sync if (tau == 0 or j % 2 == 0) else nc.scalar
                        eng.dma_start(out3[:, j, ts(tau, NT)], ot[:])
                return piece

            # ---------- filler queue ----------
            units = []          # (cost_ns, fn) or ("M", name)
            done_markers = set()

            def add_qk(kind, j, tau):
                c = qk_chain(kind, j, tau)
                for s0 in range(0, NSTEP, 3):
                    units.append((320, lambda c=c, s=s0: c(s, 3)))

            def add_tp(tc_, fo):
                units.append((120, lambda a=tc_, b=fo: transp(a, b)))

            def add_op(tau, j):
                c = op_chain(tau, j)
                units.append((426, lambda c=c: c(0, 2)))
                units.append((426, lambda c=c: c(2, 2)))

            def marker(nm):
                units.append(("M", nm))

            def pump(budget):
                while units and budget > 0:
                    u = units.pop(0)
                    if u[0] == "M":
                        done_markers.add(u[1])
                        continue
                    u[1]()
                    budget -= u[0]

            def flush(nm):
                while nm not in done_markers:
                    u = units.pop(0)
                    if u[0] == "M":
                        done_markers.add(u[1])
                        continue
                    u[1]()

            # v-chains are emitted on demand, just ahead of the AV matmuls
            # that consume them (one token-block of lookahead).
            v_done = set()

            def ensure_v(phi, i):
                if (phi, i) not in v_done:
                    v_done.add((phi, i))
                    v_chain(phi, i)(0, NSTEP)

            # ---------- attention window ----------
            # Software-pipelined emission: QK/exp for step i+1 are emitted
            # BEFORE the exp-gated AV matmuls of step i, so the PE executes
            # QK_{i+1} during exp_i instead of being stuck behind the AVs in
            # the sequencer.  Filler is pumped before the AVs for the same
            # reason (and so an AV's v-chain dependency is always emitted
            # earlier in the stream).
            def attn_window(tau, hp, budget):
                avs = [
                    psum.tile([P, 4, HD + 1], F32, tag=f"av{hh}", bufs=1,
                              padded_shape=(P, 4, P),
                              name=f"av_{hp}_{tau}_{hh}")
                    for hh in range(2)
                ]
                ets = {}

                def emit_qk(i):
                    mu, tpt = divmod(i, T // P)
                    qk = psum.tile([P, 2 * NT], F32, tag="qk", bufs=2,
                                   name=f"qk_{hp}_{tau}_{i}")
                    for hh in range(2):
                        base = hh * HD
                        fo = mu * (FG // P) + hp
                        nc.tensor.matmul(
                            qk[:, ts(hh, NT)],
                            kfeat[base:base + HD, fo, ts(tpt, P)],
                            q_sb[base:base + HD, hp, ts(tau, NT)],
                            start=True, stop=True)
                    et = ep.tile([P, 2 * NT], BF16, tag="e", bufs=3,
                                 name=f"e_{hp}_{tau}_{i}")
                    nc.scalar.activation(et[:], qk[:], EXP)
                    ets[i] = et

                def emit_av(i):
                    mu, tpt = divmod(i, T // P)
                    et = ets.pop(i)
                    for hh in range(2):
                        h = hp * 2 + hh
                        vcol = (mu * HG + h) * (HD + 1)
                        for qc in range(4):
                            nc.tensor.matmul(
                                avs[hh][:, qc, :],
                                et[:, hh * NT + qc * P:
                                   hh * NT + (qc + 1) * P],
                                vaug[:, tpt, vcol:vcol + HD + 1],
                                start=(i == 0 and qc == 0),
                                stop=(i == S // P - 1 and qc == 3),
                                skip_group_check=True)

                phis = (hp // 2, MULT + hp // 2)
                mu0, tpt0 = divmod(0, T // P)
                ensure_v(phis[mu0], tpt0)
                emit_qk(0)
                for i in range(S // P):
                    # v-chain for the NEXT step, ahead of this step's QK so
                    # the exp wait-point stays tight after the QK matmuls.
                    if i + 1 < S // P:
                        mu_, tpt_ = divmod(i + 1, T // P)
                        ensure_v(phis[mu_], tpt_)
                        emit_qk(i + 1)
                    emit_av(i)
                    pump(budget)
                for hh in range(2):
                    h = hp * 2 + hh
                    rec = ob.tile([P, 4], F32, tag="rec", bufs=2,
                                  name=f"rec_{hp}_{tau}_{hh}")
                    nc.vector.reciprocal(rec[:], avs[hh][:, :, HD])
                    for qc in range(4):
                        nc.vector.tensor_scalar(
                            a_sb[:, tau * 4 + qc, h * HD:(h + 1) * HD],
                            avs[hh][:, qc, 0:HD],
                            rec[:, qc:qc + 1],
                            None, MUL)

            # ---------- emission plan ----------
            # head: minimum work for window (0,0)'s QK stream
            qk_chain("q", 0, 0)(0, NSTEP)
            qk_chain("k", 0, 0)(0, NSTEP)
            qk_chain("k", 0, 1)(0, NSTEP)

            # filler (k/q projections, transposes, tau0 output projection)
            # in deadline order with window-entry markers; v-chains are
            # handled just-in-time by ensure_v inside the windows.  mu=1 K
            # feeds (k4..k7) are intra-window filler (QK touches them from
            # the step-7 emission).
            add_qk("k", 4, 0)
            add_qk("k", 4, 1)
            marker("w01")
            add_qk("q", 1, 0)
            add_qk("k", 1, 0)
            add_qk("k", 1, 1)
            marker("w01b")
            add_qk("k", 5, 0)           # intra (0,1)
            add_qk("k", 5, 1)
            marker("w10")
            add_qk("q", 0, 1)
            marker("w11")
            add_qk("q", 1, 1)
            add_qk("q", 2, 0)
            add_qk("k", 2, 0)
            add_qk("k", 2, 1)
            marker("w02")
            add_qk("k", 6, 0)           # intra (0,2)
            add_qk("k", 6, 1)
            add_qk("q", 3, 0)
            add_qk("k", 3, 0)
            add_qk("k", 3, 1)
            marker("w03")
            add_qk("k", 7, 0)           # intra (0,3)
            add_qk("k", 7, 1)
            marker("w12")
            add_qk("q", 2, 1)
            marker("w13")
            add_qk("q", 3, 1)

            attn_window(0, 0, budget=450)
            flush("w01b")
            attn_window(0, 1, budget=450)
            flush("w10")
            attn_window(1, 0, budget=450)
            flush("w11")
            attn_window(1, 1, budget=450)
            flush("w02")
            attn_window(0, 2, budget=450)
            flush("w03")
            attn_window(0, 3, budget=450)
            # tau0 fully drained: transposes + tau0 output projection become
            # the filler for the remaining two windows.
            for tc_ in range(4):
                for fo in range(FG // P):
                    add_tp(tc_, fo)
            for j in range(E // P):
                add_op(0, j)
            flush("w12")
            attn_window(1, 2, budget=450)
            flush("w13")
            # tau1 transposes for fo 0..2 are already unblocked (windows
            # (1,0)..(1,2) drained) — pump them inside the last window.
            for tc_ in range(4, 8):
                for fo in range(FG // P - 1):
                    add_tp(tc_, fo)
            attn_window(1, 3, budget=450)

            # tail: leftovers, last tau1 transposes, tau1 output projection
            pump(10**9)
            for tc_ in range(4, 8):
                transp(tc_, FG // P - 1)
            tags = [("mm", 2), ("mm", 2), ("qk", 2), ("qk", 2),
                    ("av0", 1), ("av1", 1)]
            for j in range(E // P):
                tg, tb = tags[j % len(tags)]
                op_chain(1, j, tg, tb)(0, FG // P)

    nc.compile()
    return nc


def _get_compiled(aug):
    if aug not in _compiled:
        _compiled[aug] = _build(aug)
    return _compiled[aug]


def _numpy_reference(hidden_states, attention_mask, Wq, bq, Wk, bk, Wv, bv,
                     Wo, bo):
    """Exact fp32 fallback (used only when attention_mask is nonzero)."""
    x = hidden_states
    q = (np.einsum("bte,fe->btf", x, Wq) + bq) * SCALE
    q = q.reshape(B, T, H, HD).transpose(0, 2, 1, 3)
    k = (np.einsum("bte,fe->btf", x, Wk) + bk).reshape(B, S, H, HD)
    k = k.transpose(0, 2, 1, 3)
    v = (np.einsum("bte,fe->btf", x, Wv) + bv).reshape(B, S, H, HD)
    v = v.transpose(0, 2, 1, 3)
    attn = np.einsum("bhtd,bhsd->bhts", q, k)
    attn = attn.reshape(B, H, T, MULT, T) + attention_mask[:, :, :, None, :]
    attn = attn.reshape(B, H, T, S)
    attn = attn - attn.max(-1, keepdims=True)
    attn = np.exp(attn)
    attn /= attn.sum(-1, keepdims=True)
    out = np.einsum("bhts,bhsd->bhtd", attn, v)
    out = out.transpose(0, 2, 1, 3).reshape(B, T, E)
    return (np.einsum("bte,fe->btf", out, Wo) + bo).astype(np.float32)


def kernel(hidden_states, attention_mask, Wq, bq, Wk, bk, Wv, bv, Wo, bo):
    import ml_dtypes

    hidden_states = np.asarray(hidden_states, dtype=np.float32)
    attention_mask = np.asarray(attention_mask, dtype=np.float32)
    Wq = np.asarray(Wq, dtype=np.float32)
    bq = np.asarray(bq, dtype=np.float32)
    Wk = np.asarray(Wk, dtype=np.float32)
    bk = np.asarray(bk, dtype=np.float32)
    Wv = np.asarray(Wv, dtype=np.float32)
    bv = np.asarray(bv, dtype=np.float32)
    Wo = np.asarray(Wo, dtype=np.float32)
    bo = np.asarray(bo, dtype=np.float32)

    if attention_mask.any():
        # The TRN2 kernel folds the (always-zero) mask away; handle the
        # general case exactly on host.
        return _numpy_reference(hidden_states, attention_mask, Wq, bq, Wk,
                                bk, Wv, bv, Wo, bo)

    aug = bool(bv.any())
    nc = _get_compiled(aug)

    bf = ml_dtypes.bfloat16
    f8 = ml_dtypes.float8_e4m3fn
    ident_np = np.eye(P, dtype=bf)

    def split8(a, smax=224.0):
        """fp8e4 (hi, lo-residual) split along a new axis 1, with a
        power-of-2 scale keeping values inside the TRN fp8e4 range."""
        m = np.abs(a).max()
        s = float(2.0 ** np.floor(np.log2(smax / m))) if m > 0 else 1.0
        hi = (a * s).astype(f8)
        lo = (a * s - hi.astype(np.float32)).astype(f8)
        return np.stack([hi, lo], axis=1), s

    in_maps = []
    for core in range(N_CORES):
        b, g = divmod(core, G)
        rows = slice(g * FG, (g + 1) * FG)
        wk_g = np.concatenate(
            [Wk[m * E + g * FG: m * E + (g + 1) * FG] for m in range(MULT)], 0)
        bk_g = np.concatenate(
            [bk[m * E + g * FG: m * E + (g + 1) * FG] for m in range(MULT)], 0)
        if aug:
            # augmented V weights/bias: per (mu, head) HD cols + ones col
            wv_g = np.zeros((E, FV), dtype=np.float32)
            bv_aug = np.zeros((FV,), dtype=np.float32)
            for m in range(MULT):
                for h in range(HG):
                    col = (m * HG + h) * (HD + 1)
                    r0 = m * E + g * FG + h * HD
                    wv_g[:, col:col + HD] = Wv[r0:r0 + HD].T
                    bv_aug[col:col + HD] = bv[r0:r0 + HD]
                    bv_aug[col + HD] = 1.0
        else:
            wv_g = np.concatenate(
                [Wv[m * E + g * FG: m * E + (g + 1) * FG]
                 for m in range(MULT)], 0).T
        # row order (p, ko, half): [E, 2, F] -> [KO, P, 2, F] -> [P, KO, 2, F]
        def pkt(a8):
            n = a8.shape[-1]
            return np.ascontiguousarray(
                a8.reshape(KO, P, 2, n).transpose(1, 0, 2, 3)
            ).reshape(2 * E, n)

        x8, s_x = split8(hidden_states[b].T, 192.0)
        wq8, s_q = split8((Wq[rows] * SCALE).T)
        wk8, s_k = split8(wk_g.T)
        wv8, s_v = split8(np.ascontiguousarray(wv_g))
        x8, wq8, wk8, wv8 = pkt(x8), pkt(wq8), pkt(wk8), pkt(wv8)
        wsc_np = np.broadcast_to(
            np.array([1 / (s_x * s_q), 1 / (s_x * s_k), 1 / (s_x * s_v), 0],
                     np.float32), (P, 4)).copy()
        in_maps.append({
            "xT": x8,
            "wq": wq8,
            "wk": wk8,
            "wv": wv8,
            "wo": Wo[:, g * FG:(g + 1) * FG].T.astype(bf),
            "wsc": wsc_np,
            "bq": np.ascontiguousarray(bq[rows] * SCALE),
            "bk": np.ascontiguousarray(bk_g),
            "bo": bo if g == 0 else np.zeros_like(bo),
            "ident": ident_np,
        })
        if aug:
            in_maps[-1]["bv"] = (bv_aug * s_v * s_x).astype(bf)
            in_maps[-1]["ones"] = np.ones(P, dtype=bf)

    res = bass_utils.run_bass_kernel_spmd(
        nc, in_maps, core_ids=list(range(N_CORES)))

    final = np.empty((B, T, E), dtype=np.float32)
    for b in range(B):
        acc = (res.results[G * b]["out"].astype(np.float32)
               + res.results[G * b + 1]["out"].astype(np.float32))
        final[b] = acc.T
    return final
